# revision 14
# baseline (speedup 1.0000x reference)
"""CLSTM-with-projection TRN2 kernel, 8-core tensor-parallel, v2 (receiver-m).

Structure vs v1 (Wf-gather baseline): comm is IDENTICAL (each core
broadcasts its 256-col bf16 mpre^T slice into the peers' gb slots, one
remote_dma_broadcast per step). The change is post-wire: instead of the
K=2048 gather (32 matmuls of ap=512: gates += mpre @ (w_proj @ w_m)),
each core computes

    m(t-1) = mpre(t-1) @ w_proj          16 mm ap=512  -> pmo psum
    m_sb   = bf16(pmo)                   ACT copy
    m^T    = transpose(m_sb chunks)      4 tr          -> ptr2 psum
    sTi2   = bf16(ptr2)                  DVE copy
    gates += m^T.T @ w_m                 8 mm ap=512 (K=512)

i.e. 28 matmul-class instructions and ~12.5K PE cycles post-wire vs 32
and 16K+proj. The old ppj projection (16 mm) disappears entirely: the
output slice is read straight out of pmo (full m) with a register-offset
DVE copy.

Wire probes (sendw knobs on v1) showed broadcast cost rises ~21ns per
extra bf16 column above 256 — shipping s^T/m partials (512 cols) costs
+5.5us/step, which is why the sender-side-partial variant loses; the
256-col mpre^T payload is information-minimal and stays.
"""

import numpy as np
import ml_dtypes

import concourse.bass as bass
import concourse.mybir as mybir
from concourse import bacc, library_config

B = 128
T = 128
DIN = 512
U = 2048
PJ = 512
G = 4 * U
NC = 8
UL = U // NC  # 256
GL = G // NC  # 1024
PJL = PJ // NC  # 64
F32 = mybir.dt.float32
F32R = mybir.dt.float32r
BF16 = mybir.dt.bfloat16
SIGF = mybir.ActivationFunctionType.Sigmoid
TANHF = mybir.ActivationFunctionType.Tanh
COPYF = mybir.ActivationFunctionType.Copy
BF16NP = ml_dtypes.bfloat16


def r(ap):
    return ap.bitcast(F32R)


def make_milestones(TS, NB, NWARM=0, WA=2, WB=3, MCN=16, NGATES=8, TRCH=True, S0DV=True):
    M = {}
    NTR = 4 if TRCH else 0
    # --- PE chain ---
    n = 8 + NB  # xproj(0), groups stopped
    M["pe_xp0"] = n
    M["pe_gn0_0"] = 4 + NB // 2
    M["pe_g0"] = n
    for t in range(TS):
        if t >= 1:
            n += MCN
            M[f"pe_mc{t - 1}"] = n  # m-calc of m(t-1)
            n += WA  # hop-filling warms
            n += NTR
            M[f"pe_mt{t - 1}"] = n  # transposes m_sb -> ptr2
            n += WB
            n += NGATES // 2
            M[f"pe_gn0_{t}"] = n
            n += NGATES // 2
            M[f"pe_g{t}"] = n
        if t + 1 < TS:
            n += 8 + NB
            M[f"pe_xp{t + 1}"] = n
        n += 2
        M[f"pe_tr{t}"] = n  # mp transposes
        n += NWARM
    # final m-calc + output for t = TS-1
    n += MCN
    M[f"pe_mc{TS - 1}"] = n
    # --- ACT chain: [mcp(t-1)] sfj sio th per step (f-bias folded into b_s) ---
    n = 0
    NMCP = 1 if TRCH else 0
    for t in range(TS):
        if t >= 1:
            n += NMCP
            M[f"ac_mcp{t - 1}"] = n
        n += 1
        M[f"ac_fj_{t}"] = n
        n += 1
        M[f"ac_io_{t}"] = n
        n += 1
        M[f"ac_th{t}"] = n
    # --- DVE chain ---
    n = 2 + (2 if NB else 0) + (0 if TRCH else 1)
    M["dv_init"] = n
    NST2 = 1 if TRCH else 0
    for t in range(TS):
        if t >= 1:
            n += NST2
            M[f"dv_st2_{t - 1}"] = n  # sTi2 <- ptr2
            n += 1
            M[f"dv_mo{t - 1}"] = n  # mo <- pmo slice
        n += 3  # tB, tA, c
        M[f"dv_c{t}"] = n
        n += 1
        M[f"dv_mp{t}"] = n
        n += 1 if S0DV else 0
        M[f"dv_s0_{t}"] = n  # gb own-slot copy (merged 256-col)
    n += 1
    M[f"dv_mo{TS - 1}"] = n
    return M


def build(ts=T, knobs=(), has_bias=True):
    NWARM = 26
    WA, WB = 1, 1
    PEFREE = "pefree" in knobs   # timing diag: mcalc skips the rsem wait
    NOMC = "nomcalc" in knobs    # timing diag: 2-chunk mcalc (WRONG results)
    NOGT = "nogates" in knobs    # timing diag: no m-gates (WRONG results)
    NOTR = "notr" in knobs       # timing diag: no mcp/tr4/st2 chain (WRONG results)
    NOTRIG = "notrig" in knobs   # timing diag: trigger skips dv_s0 wait (WRONG)
    POOLS0 = "pools0" in knobs   # gb own-slot copy on Pool (removes DVE handoff)
    BIGWARM = "smallwarm" not in knobs  # ap=512 keep-warm fill (measured best)
    D16 = "d16" in knobs         # 16-slot rdests (1 DMA engine per dest)
    SENDW = UL  # broadcast payload cols (timing diag below 256: WRONG results)
    for k in knobs:
        if k.startswith("warm"):
            NWARM = int(k[4:])
        elif k.startswith("ha"):
            WA = int(k[2:])
        elif k.startswith("hb"):
            WB = int(k[2:])
        elif k.startswith("sw"):
            SENDW = int(k[2:])
    NB = 2 if has_bias else 0
    TS = ts
    MCN = 2 if NOMC else 16
    NGATES = 0 if NOGT else 8
    TRCH = not NOTR
    M = make_milestones(TS, NB, NWARM, WA, WB, MCN, NGATES, TRCH, S0DV=not POOLS0)
    nc = bacc.Bacc("TRN2", target_bir_lowering=False, debug=False, num_devices=NC)

    xT_d = nc.declare_dram_parameter("xT", [DIN, TS * B], F32R, isOutput=False)
    wx_d = nc.declare_dram_parameter("wx_s", [DIN, GL], F32R, isOutput=False)
    wm_d = nc.declare_dram_parameter("wm_s", [PJ, GL], BF16, isOutput=False)
    wpj_d = nc.declare_dram_parameter("wpj2", [U, PJ], BF16, isOutput=False)
    b_d = nc.declare_dram_parameter("b_s", [1, GL], F32R, isOutput=False)
    id_d = nc.declare_dram_parameter("ident", [128, 128], F32R, isOutput=False)
    idh_d = nc.declare_dram_parameter("identh", [128, 128], BF16, isOutput=False)
    out_d = nc.declare_dram_parameter("out_m", [TS * B, PJL], F32, isOutput=True)

    from contextlib import ExitStack

    es = ExitStack()
    with es:
        sb = lambda n_, sh: es.enter_context(nc.sbuf_tensor(n_, sh, F32))
        sbr = lambda n_, sh: es.enter_context(nc.sbuf_tensor(n_, sh, F32R))
        sbh = lambda n_, sh: es.enter_context(nc.sbuf_tensor(n_, sh, BF16))
        ps = lambda n_, sh: es.enter_context(nc.psum_tensor(n_, sh, F32))
        sem = lambda n_: es.enter_context(nc.semaphore(n_))
        wx_sb = sbr("wx_sb", [128, 4 * GL])
        wm_sb = sbh("wm_sb", [128, 4 * GL])
        wpj_sb = sbh("wpj_sb", [128, 16 * PJ])
        bsb = sbr("bsb", [1, GL])
        ones = sb("ones", [1, 128])
        ones_r = sbr("ones_r", [1, 128])
        idt = sbr("idt", [128, 128])
        idh = sbh("idh", [128, 128])
        g0 = sbh("g0", [128, U])
        g1 = sbh("g1", [128, U])
        m_sb = sbh("m_sb", [128, PJ])
        sTi2 = sbh("sTi2", [128, PJ])
        sg = sb("sg", [128, GL])
        tnh = sb("tnh", [128, UL])
        c0t = sb("c0", [128, UL])
        c1t = sb("c1", [128, UL])
        tA = sb("tA", [128, UL])
        tB = sb("tB", [128, UL])
        mp = sbh("mp", [128, UL])
        xl = sbr("xl", [128, 2 * DIN])
        mo = sb("mo", [128, 2 * PJL])
        pg0 = ps("pg0", [128, GL])
        pg1 = ps("pg1", [128, GL])
        ptr = es.enter_context(nc.psum_tensor("ptr", [128, 256], BF16))
        pmo = ps("pmo", [128, PJ])
        ptr2 = es.enter_context(nc.psum_tensor("ptr2", [128, PJ], BF16))
        scr = ps("scr", [128, 512])
        rsem0 = sem("rsem0")
        rsem1 = sem("rsem1")
        lsem0 = sem("lsem0")
        lsem1 = sem("lsem1")
        psem = sem("psem")
        ldsem = sem("ldsem")
        xla = sem("xla")
        xlb = sem("xlb")
        os0 = sem("os0")
        os1 = sem("os1")
        pcp = sem("pcp")
        pchn = sem("pchn")
        achn = sem("achn")
        dvch = sem("dvch")
        block = es.enter_context(nc.Block())

        gat = [g0, g1]
        rsems = [rsem0, rsem1]
        lsems = [lsem0, lsem1]
        pgs = [pg0, pg1]
        cts = [c0t, c1t]
        xlsems = [xla, xlb]
        osems = [os0, os1]

        LD0 = 16 * (4 + 4 + 16 + 1 + 1 + 1)

        # ---------------- PE ----------------
        @block.tensor
        def _(pe):
            cnt = [0]

            def mm(*a, **kw):
                pe.matmul(*a, **kw).then_inc(pchn, 1)
                cnt[0] += 1

            def tr(*a):
                pe.transpose(*a).then_inc(pchn, 1)
                cnt[0] += 1

            def warm(k):
                # fine-grained keep-warm: ap=64 bf16 (~107ns each) so the
                # fill can't overshoot the critical path by more than ~0.1us
                for _ in range(k):
                    if BIGWARM:
                        mm(scr[:, :], r(wx_sb[:, 0:128]), r(wx_sb[:, 0:512]),
                           start=True, stop=True)
                    else:
                        mm(scr[:, 0:64], wm_sb[:, 0:128], wm_sb[:, 0:64],
                           start=True, stop=True)

            def xproj(t):
                pe.wait_ge(xlsems[t % 2], 64 * (t // 2 + 1))
                pg = pgs[t % 2]
                last = t == 0
                for nt in range(2):
                    if has_bias:
                        mm(pg[:, nt * 512 : (nt + 1) * 512], ones_r[:, :],
                           bsb[:, nt * 512 : (nt + 1) * 512], start=True, stop=False)
                    for kc in range(4):
                        mm(pg[:, nt * 512 : (nt + 1) * 512],
                           r(xl[:, (t % 2) * DIN + kc * 128 : (t % 2) * DIN + (kc + 1) * 128]),
                           r(wx_sb[:, kc * GL + nt * 512 : kc * GL + (nt + 1) * 512]),
                           start=(kc == 0 and not has_bias),
                           stop=((last or NOGT) and kc == 3))
                if last:
                    assert cnt[0] == M["pe_g0"]

            def mcalc(u):
                # m(u) = mpre(u) @ w_proj from the gathered gb slices.
                # dv_s0_u transitively implies: dv_mo{u-1} (pmo free, DVE
                # chain order) and ac_mcp{u-1} (via st2(u-1) <- pe_mt{u-1}
                # <- ac_mcp{u-1}), so 2 waits suffice.
                if not PEFREE:
                    pe.wait_ge(rsems[u % 2], 14 * (u // 2 + 1))
                if POOLS0:
                    pe.wait_ge(pcp, u + 1)  # pool gb-copy chain
                else:
                    pe.wait_ge(dvch, M[f"dv_s0_{u}"])
                gb = gat[u % 2]
                for cu in range(MCN):
                    mm(pmo[:, :], gb[:, cu * 128 : (cu + 1) * 128],
                       wpj_sb[:, cu * PJ : (cu + 1) * PJ],
                       start=(cu == 0), stop=(cu == MCN - 1))
                assert cnt[0] == M[f"pe_mc{u}"]

            pe.wait_ge(ldsem, LD0)
            if has_bias:
                pe.wait_ge(dvch, 2)  # ones ready (f32r)
            # pre-loop: gates(0) = bias + xproj only (m(-1) == 0)
            xproj(0)
            for t in range(TS):
                pg = pgs[t % 2]
                if t >= 1:
                    u = t - 1
                    mcalc(u)
                    warm(WA)  # fill the ACT-copy hop
                    if TRCH:
                        # m^T(u): transposes of m_sb chunks. ptr2-free
                        # (dv_st2{u-1}) is implied by mcalc's dv_s0_u wait.
                        pe.wait_ge(achn, M[f"ac_mcp{u}"])
                        for c in range(4):
                            tr(ptr2[:, c * 128 : (c + 1) * 128],
                               m_sb[:, c * 128 : (c + 1) * 128], idh[:, :])
                    assert cnt[0] == M[f"pe_mt{u}"]
                    warm(WB)  # fill the DVE-copy hop
                    if not NOGT:
                        # gates(t): m^T(t-1) @ w_m completes the psum groups
                        if TRCH:
                            pe.wait_ge(dvch, M[f"dv_st2_{u}"])
                        for nt in range(2):
                            for kc in range(4):
                                mm(pg[:, nt * 512 : (nt + 1) * 512],
                                   sTi2[:, kc * 128 : (kc + 1) * 128],
                                   wm_sb[:, kc * GL + nt * 512 : kc * GL + (nt + 1) * 512],
                                   start=False, stop=(kc == 3))
                            assert cnt[0] == M[f"pe_gn0_{t}" if nt == 0 else f"pe_g{t}"]
                if t + 1 < TS:
                    # next step's xproj: overlaps this step's ACT/DVE tail
                    if t >= 1:
                        pe.wait_ge(achn, M[f"ac_io_{t - 1}"])  # pg slot free
                    xproj(t + 1)
                    assert cnt[0] == M[f"pe_xp{t + 1}"]
                # transpose mp(t) -> ptr (waits the ACT/DVE tail of step t;
                # ptr-free dv_s0_{t-1} is implied: dv_s0_{t-1} < dv_mp_t)
                pe.wait_ge(dvch, M[f"dv_mp{t}"])
                if POOLS0 and t >= 1:
                    pe.wait_ge(pcp, t)  # ptr freed by pool gb-copy(t-1)
                for h in range(2):
                    tr(ptr[:, h * 128 : (h + 1) * 128], mp[:, h * 128 : (h + 1) * 128],
                       idh[:, :])
                assert cnt[0] == M[f"pe_tr{t}"]
                warm(NWARM)
            # final m-calc for the last output step
            mcalc(TS - 1)

        # ---------------- ACT ----------------
        # per-core gate column order [f j i o] (host-permuted):
        #   f: sg[0:256] (bias 1), j: [256:512], i: [512:768], o: [768:1024]
        @block.scalar
        def _(a):
            cnt = [0]

            def act(out, in_, func, bias=0.0):
                a.wait_ge(achn, cnt[0])  # serialize same-engine
                a.activation(out, in_, func, bias=bias).then_inc(achn, 1)
                cnt[0] += 1

            for t in range(TS):
                if t >= 1 and TRCH:
                    u = t - 1
                    # m_sb <- pmo (psum f32 -> sbuf bf16); m_sb-consumed
                    # (pe_mt{u-1}) is implied: pe_mt{u-1} < pe_mc{u}
                    a.wait_ge(pchn, M[f"pe_mc{u}"])
                    act(m_sb[:, :], pmo[:, :], COPYF)
                    assert cnt[0] == M[f"ac_mcp{u}"]
                a.wait_ge(pchn, M[f"pe_gn0_{t}"] if t >= 1 else M["pe_gn0_0"])
                if t >= 1:
                    a.wait_ge(dvch, M[f"dv_mp{t - 1}"])  # sg reuse (tail read)
                pg = pgs[t % 2]
                act(sg[:, 0:512], pg[:, 0:512], SIGF)  # f (+1 in b_s), j
                assert cnt[0] == M[f"ac_fj_{t}"]
                a.wait_ge(pchn, M[f"pe_g{t}"])
                act(sg[:, 512:1024], pg[:, 512:1024], SIGF)  # i, o
                assert cnt[0] == M[f"ac_io_{t}"]
                a.wait_ge(dvch, M[f"dv_c{t}"])
                act(tnh[:, :], cts[t % 2][:, :], TANHF)
                assert cnt[0] == M[f"ac_th{t}"]

        # ---------------- DVE ----------------
        @block.vector
        def _(v):
            cnt = [0]

            def op(fn, *a, **kw):
                v.wait_ge(dvch, cnt[0])  # serialize same-engine
                fn(*a, **kw).then_inc(dvch, 1)
                cnt[0] += 1

            offv = v.partition_id() * UL
            offm = v.partition_id() * PJL
            if has_bias:
                op(v.memset, ones[:, :], 1.0)
                op(v.tensor_copy, ones_r[:, :], ones[:, :])
            op(v.memset, cts[0][:, :], 0.0)
            op(v.memset, cts[1][:, :], 0.0)
            if not TRCH:
                op(v.memset, sTi2[:, :], 0.0)
            assert cnt[0] == M["dv_init"]
            for t in range(TS):
                if t >= 1:
                    u = t - 1
                    if TRCH:
                        # sTi2 <- ptr2 (m^T(u), psum f32 -> sbuf bf16)
                        v.wait_ge(pchn, M[f"pe_mt{u}"])
                        op(v.tensor_copy, sTi2[:, :], ptr2[:, :])
                        assert cnt[0] == M[f"dv_st2_{u}"]
                    else:
                        v.wait_ge(pchn, M[f"pe_mt{u}"])
                    # mo <- own output slice of pmo (= m(u))
                    if u >= 2:
                        v.wait_ge(osems[u % 2], 16 * (u // 2))
                    op(v.tensor_copy, mo[:, (u % 2) * PJL : (u % 2 + 1) * PJL],
                       pmo[:, bass.ds(offm, PJL)])
                    assert cnt[0] == M[f"dv_mo{u}"]
                v.wait_ge(achn, M[f"ac_fj_{t}"])
                op(v.tensor_mul, tB[:, :], sg[:, 0:256], cts[(t + 1) % 2][:, :])
                v.wait_ge(achn, M[f"ac_io_{t}"])
                op(v.tensor_mul, tA[:, :], sg[:, 256:512], sg[:, 512:768])
                op(v.tensor_add, cts[t % 2][:, :], tA[:, :], tB[:, :])
                assert cnt[0] == M[f"dv_c{t}"]
                v.wait_ge(achn, M[f"ac_th{t}"])
                if t >= 1:
                    v.wait_ge(pchn, M[f"pe_tr{t - 1}"])  # mp consumed
                op(v.tensor_mul, mp[:, :], sg[:, 768:1024], tnh[:, :])
                assert cnt[0] == M[f"dv_mp{t}"]
                if not POOLS0:
                    v.wait_ge(pchn, M[f"pe_tr{t}"])
                    if t >= 2:
                        v.wait_ge(lsems[t % 2], 16 * (t // 2))
                    gb = gat[t % 2]
                    op(v.tensor_copy, gb[:, bass.ds(offv, 256)], ptr[:, 0:256])
                assert cnt[0] == M[f"dv_s0_{t}"]
            u = TS - 1
            v.wait_ge(pchn, M[f"pe_mc{u}"])
            v.wait_ge(osems[u % 2], 16 * (u // 2))
            op(v.tensor_copy, mo[:, (u % 2) * PJL : (u % 2 + 1) * PJL],
               pmo[:, bass.ds(offm, PJL)])
            assert cnt[0] == M[f"dv_mo{u}"]

        # ---------------- Pool: remote broadcast ----------------
        @block.gpsimd
        def _(g):
            g.load_library(library_config.remote_dma)
            offp = g.partition_id() * UL
            if D16:
                rdests = [None] + [(0, j) for j in range(1, NC)] + [None] * 8
            else:
                rdests = [None] + [(0, j) for j in range(1, NC)]

            def descgen(t):
                gb = gat[t % 2]
                g.remote_dma_broadcast(
                    gb[:, bass.ds(offp, SENDW)],
                    gb[:, bass.ds(offp, SENDW)],
                    rsems[t % 2],
                    lsems[t % 2],
                    rdests=rdests,
                ).then_inc(psem, 1)

            PREBANK = min(6, TS)
            for t0 in range(PREBANK):
                descgen(t0)
            for t in range(TS):
                g.wait_ge(psem, t + 1)
                if POOLS0:
                    # gb own-slot copy on Pool, then trigger (no cross-engine hop)
                    g.wait_ge(pchn, M[f"pe_tr{t}"])
                    if t >= 2:
                        g.wait_ge(lsems[t % 2], 16 * (t // 2))
                    g.tensor_copy(gat[t % 2][:, bass.ds(offp, 256)],
                                  ptr[:, 0:256]).then_inc(pcp, 1)
                elif not NOTRIG:
                    g.wait_ge(dvch, M[f"dv_s0_{t}"])
                g.trigger_dma(count=1)
                if t + PREBANK < TS:
                    descgen(t + PREBANK)

        # ---------------- SP ----------------
        @block.sync
        def _(s):
            for kc in range(4):
                s.dma_start(wx_sb[:, kc * GL : (kc + 1) * GL],
                            wx_d[kc * 128 : (kc + 1) * 128, :]).then_inc(ldsem, 16)
            for kc in range(4):
                s.dma_start(wm_sb[:, kc * GL : (kc + 1) * GL],
                            wm_d[kc * 128 : (kc + 1) * 128, :]).then_inc(ldsem, 16)
            for cu in range(16):
                s.dma_start(wpj_sb[:, cu * PJ : (cu + 1) * PJ],
                            wpj_d[cu * 128 : (cu + 1) * 128, :]).then_inc(ldsem, 16)
            s.dma_start(bsb[:, :], b_d[:, :]).then_inc(ldsem, 16)
            s.dma_start(idt[:, :], id_d[:, :]).then_inc(ldsem, 16)
            s.dma_start(idh[:, :], idh_d[:, :]).then_inc(ldsem, 16)
            # loop: xl loads(t) first, then out store(u = t-2)
            for t in range(TS + 2):
                if t < TS:
                    if t >= 2:
                        s.wait_ge(pchn, M[f"pe_g{t - 2}"])
                    for kc in range(4):
                        s.dma_start(
                            xl[:, (t % 2) * DIN + kc * 128 : (t % 2) * DIN + (kc + 1) * 128],
                            xT_d[kc * 128 : (kc + 1) * 128, t * B : (t + 1) * B],
                        ).then_inc(xlsems[t % 2], 16)
                if t >= 2:
                    u = t - 2
                    s.wait_ge(dvch, M[f"dv_mo{u}"])
                    s.dma_start(
                        out_d[u * B : (u + 1) * B, :],
                        mo[:, (u % 2) * PJL : (u % 2 + 1) * PJL],
                    ).then_inc(osems[u % 2], 16)
            s.wait_ge(os0, 16 * (TS - TS // 2))
            s.wait_ge(os1, 16 * (TS // 2))

    nc.compile()
    return nc


# ---------------------------------------------------------------------------
# SPMD runner (inlined; modeled on concourse.bass2jax.run_bass_via_pjrt)
# ---------------------------------------------------------------------------
import time
import jax
from jax.sharding import Mesh, PartitionSpec
from jax.experimental.shard_map import shard_map
from concourse.bass2jax import (
    _bass_exec_p,
    install_neuronx_cc_hook,
    partition_id_tensor,
)


class SpmdRunner:
    def __init__(self, nc, n_cores):
        install_neuronx_cc_hook()
        self.nc = nc
        self.n_cores = n_cores
        partition_name = nc.partition_id_tensor.name if nc.partition_id_tensor else None
        in_names, out_names, out_avals, zero_outs = [], [], [], []
        for alloc in nc.m.functions[0].allocations:
            if not isinstance(alloc, mybir.MemoryLocationSet):
                continue
            name = alloc.memorylocations[0].name
            if alloc.kind == "ExternalInput":
                if name != partition_name:
                    in_names.append(name)
            elif alloc.kind == "ExternalOutput":
                out_names.append(name)
                shape = tuple(alloc.tensor_shape)
                dtype = mybir.dt.np(alloc.dtype)
                out_avals.append(jax.core.ShapedArray(shape, dtype))
                zero_outs.append(np.zeros(shape, dtype))
        self.in_names = list(in_names)
        self.out_names = out_names
        self.out_avals = out_avals
        self.zero_outs = zero_outs
        n_params = len(in_names)
        all_in_names = in_names + out_names
        if partition_name is not None:
            all_in_names.append(partition_name)

        def _body(*args):
            operands = list(args)
            if partition_name is not None:
                operands.append(partition_id_tensor())
            outs = _bass_exec_p.bind(
                *operands,
                out_avals=tuple(out_avals),
                in_names=tuple(all_in_names),
                out_names=tuple(out_names),
                lowering_input_output_aliases=(),
                sim_require_finite=True,
                sim_require_nnan=True,
                nc=nc,
            )
            return tuple(outs)

        devices = jax.devices()[:n_cores]
        self.mesh = Mesh(np.asarray(devices), ("core",))
        in_specs = (PartitionSpec("core"),) * (n_params + len(out_names))
        out_specs = (PartitionSpec("core"),) * len(out_names)
        self.fn = jax.jit(
            shard_map(
                _body,
                mesh=self.mesh,
                in_specs=in_specs,
                out_specs=out_specs,
                check_rep=False,
            ),
            keep_unused=True,
        )

    def put_inputs(self, in_maps):
        """Device-put per-core inputs (list of dicts) + zero outputs."""
        n = self.n_cores
        sh = jax.sharding.NamedSharding(self.mesh, PartitionSpec("core"))
        args = []
        for name in self.in_names:
            concat = np.concatenate([np.asarray(in_maps[c][name]) for c in range(n)], axis=0)
            args.append(jax.device_put(concat, sh))
        for z in self.zero_outs:
            concat = np.zeros((n * z.shape[0], *z.shape[1:]), z.dtype)
            args.append(jax.device_put(concat, sh))
        return args

    def run(self, args):
        outs = self.fn(*args)
        jax.block_until_ready(outs)
        return outs

    def chain_run(self, args, n_chain):
        """Issue n_chain executions back-to-back, each feeding its output
        into the next one's donated output buffer (forces sequential device
        execution), block on the last. Used for marginal-cost timing."""
        a = list(args)
        out = None
        for _ in range(n_chain):
            outs = self.fn(*a)
            out = outs[0]
            a[len(self.in_names)] = out
        jax.block_until_ready(out)
        return out

    def results(self, outs):
        res = []
        for c in range(self.n_cores):
            d = {}
            for i, name in enumerate(self.out_names):
                d[name] = np.asarray(outs[i]).reshape(
                    self.n_cores, *self.out_avals[i].shape
                )[c]
            res.append(d)
        return res

    def time_it(self, args, n_warm=2, n_rep=10):
        for _ in range(n_warm):
            self.run(args)
        ts = []
        for _ in range(n_rep):
            t0 = time.perf_counter()
            self.run(args)
            ts.append(time.perf_counter() - t0)
        return min(ts), sorted(ts)[len(ts) // 2]

    def marginal_exec_time(self, args, n1=1, n2=65, reps=12):
        """Per-execution device time: slope of wall time vs chain length.
        The ~80ms axon dispatch floor cancels in the difference; pipelined
        dispatch means the extra chained executions cost only their actual
        on-device time (plus per-call RPC processing, a few 100us)."""
        self.chain_run(args, 2)
        t1s, t2s = [], []
        for _ in range(reps):
            t0 = time.perf_counter()
            self.chain_run(args, n1)
            t1s.append(time.perf_counter() - t0)
        for _ in range(reps):
            t0 = time.perf_counter()
            self.chain_run(args, n2)
            t2s.append(time.perf_counter() - t0)
        return (min(t2s) - min(t1s)) / (n2 - n1), min(t1s), min(t2s)



_CACHE = {}


def _prep_inputs(x, w, b, w_proj, ts):
    x_tm = np.ascontiguousarray(
        np.swapaxes(np.asarray(x, np.float32), 0, 1).reshape(ts * B, DIN)
    )
    xT = np.ascontiguousarray(x_tm.T)
    w = np.asarray(w, np.float32)
    b = np.asarray(b, np.float32)
    w_proj = np.asarray(w_proj, np.float32)
    ident = np.eye(128, dtype=np.float32)
    identh = np.eye(128, dtype=BF16NP)
    wpjh = np.ascontiguousarray(w_proj.astype(BF16NP))
    in_maps = []
    for k in range(NC):
        # per-core gate column order [f j i o] (gq quadrants 2,1,0,3)
        cols = np.concatenate(
            [np.arange(gq * U + k * UL, gq * U + (k + 1) * UL) for gq in (2, 1, 0, 3)]
        )
        in_maps.append(
            {
                "xT": xT,
                "wx_s": np.ascontiguousarray(w[:DIN, cols]),
                "wm_s": np.ascontiguousarray(w[DIN:, cols].astype(BF16NP)),
                "wpj2": wpjh,
                "b_s": np.ascontiguousarray(
                    (b[cols] + np.repeat([1.0, 0, 0, 0], UL).astype(np.float32))[None, :]
                ),
                "ident": ident,
                "identh": identh,
            }
        )
    return in_maps


def kernel(x, cnn_x, w_char, b_char, w, b, w_proj):
    ts = x.shape[1]
    key = (ts, True)
    if key not in _CACHE:
        _CACHE[key] = SpmdRunner(build(ts, has_bias=True), NC)
    run = _CACHE[key]
    in_maps = _prep_inputs(x, w, b, w_proj, ts)
    args = run.put_inputs(in_maps)
    outs = run.run(args)
    res = run.results(outs)
    out = np.empty((B, ts, PJ), np.float32)
    for k in range(NC):
        om = res[k]["out_m"].reshape(ts, B, PJL)
        out[:, :, k * PJL : (k + 1) * PJL] = np.swapaxes(om, 0, 1)
    return out
